# revision 23
# baseline (speedup 1.0000x reference)
"""Trainium2 Bass kernel for the CoOccurrenceEncoder pairwise-MLP problem.

Reference computation (per batch b of 4, N=512 nodes, d=128):
    hi = x @ W1[:d]          # [N, d]
    hj = x @ W1[d:]          # [N, d]
    h  = relu(hi[:,None,:] + hj[None,:,:] + b1)       # [N, N, d]
    h2 = relu(h @ W2 + b2)                            # [N, N, 64]
    out = sigmoid((h2 @ W3 + b3)[..., 0])             # [N, N]

Sharding: 8 cores; core c handles batch c//2, i-rows [256*(c%2), 256*(c%2)+256).
Each core holds its batch's full hj [d=128 partitions, N=512] in SBUF (bf16) and
streams 256 i-rows; weights are tiny and replicated.

Per-core dataflow (d=128 on partitions everywhere):
  stage1 (DVE, bf16 2x_1P): S_i = relu(hj + (hi_i + b1)) via one dual-op
          tensor_scalar (add per-partition vector, max 0) per row -> SBUF bf16.
          Every GP_MOD-th row runs on GPSIMD instead to offload DVE.
  stage2 (PE): stationary [W2 | W2] (128x128 bf16); a row PAIR runs as two
          column-tiled matmuls that co-start -> h2 fp32 [128, 2*512] PSUM
  stage2b (ACT): relu(h2 + b2) PSUM -> SBUF bf16 (1x, PSUM-source bound);
          every DVE_RELU_PERIOD-th iteration's relu runs on DVE instead.
  stage3 (PE, 4x column-tiled): pair processed at step k (within a 64-pair
          group) targets col strip k%4 (psum partitions 32*(k%4)..+32), slot
          t=k//4 via the 32-wide stationary window zwide[:, 30-2t:62-2t]
          whose W3 pair sits at strip-relative cols 2t,2t+1. Rows are
          PERMUTED so that the pair processed at step k is pi(k) =
          16*(k%4) + k//4 -- its logits land exactly at psum partitions
          2*pi(k), 2*pi(k)+1, i.e. the identity layout. Four consecutive
          stage-3 matmuls hit 4 disjoint strips and co-start on the PE
          (4x col tiling), quartering stage-3 PE time vs the 128-wide
          zero-padded scheme.
  stage4 (ACT): one sigmoid(logits + b3) [128,512] per 64 pairs -> one
          contiguous [128,512] HWDGE DMA to HBM.
"""

import numpy as np
import ml_dtypes

import concourse.bass as bass
import concourse.mybir as mybir
import concourse.tile as tile
from concourse import bacc
from concourse.bass_utils import run_bass_kernel_spmd

F32 = mybir.dt.float32
BF16 = mybir.dt.bfloat16

D = 128          # feature dim (= partitions)
N = 512          # nodes per batch
B = 4            # batches
NCORES = 8
ROWS = 256       # i-rows per core
PAIRS = ROWS // 2
ZW = 62          # zwide stationary width

# iterations >= DVE_RELU_FROM run their stage2-relu on DVE instead of ACT.
# Mid-stream DVE relus head-of-line-block stage-1 rows (measured 10us PE
# stalls), but TAIL iterations are safe: DVE finishes its 256 stage-1 rows
# ~13us before ACT drains, so the last few relus ride DVE's idle tail and
# directly shorten the ACT critical path (~1.54us each).
# MEASURED: each non-final DVE relu precedes later iterations' stage-1 rows
# in the DVE FIFO and waits on PE mid-stream -> pushes DVE's stream end out
# ~2-3us apiece (FROM=38 regressed to 93.8us). Only the last iteration's
# relu (after ALL rows) rides the idle tail cleanly.
DVE_RELU_FROM = 42
# stage-1 rows with ridx % GP_MOD == GP_MOD//2 run on GPSIMD (0 = none).
# MEASURED: gpsimd tensor_scalar [128,512] = 7530ns steady-state AND its SBUF
# port contention doubles concurrent DVE op durations. Unusable.
GP_MOD = 0
# stage-1 rows with ridx % ACT_S1_MOD == ACT_S1_MOD//2 run on ACT (0 = none).
ACT_S1_MOD = 0


def _proc_pair(k):
    """Pair processed at step k: 64-pair groups, strip-interleaved so 4
    consecutive stage-3 matmuls target 4 disjoint 32-col PE strips."""
    g, k2 = divmod(k, 64)
    return 64 * g + 16 * (k2 % 4) + k2 // 4


def build_nc():
    # Bacc (not plain Bass): its compile() runs move_matmul_waits_to_ldweights
    # + generate_event_semaphores, needed to satisfy TRN2's 1-wait-per-matmul
    # hardware constraint.
    nc = bacc.Bacc("TRN2")

    # wpack = [w1b | w1a | w2dup | zwide] along free; bpack = [b1 | b2dup | b3]
    # zwide [128, 62]: col 30 = [W3;0], col 31 = [0;W3], zeros elsewhere. The
    # stage-3 stationary for slot t is the 32-col window zwide[:, 30-2t:62-2t]:
    # its W3 pair lands at strip-relative cols 2t,2t+1 and every other column
    # is zero, so each matmul writes its whole 32-partition strip (slot 0 with
    # start=True clears that strip's has_written bits; later slots accumulate
    # +0 everywhere except their own 2 partitions).
    xT_d = nc.dram_tensor("xT", [D, N], BF16, kind="ExternalInput")
    xTi_d = nc.dram_tensor("xTi", [D, ROWS], BF16, kind="ExternalInput")
    wpack_d = nc.dram_tensor("wpack", [D, 3 * D + ZW], BF16, kind="ExternalInput")
    bpack_d = nc.dram_tensor("bpack", [D, 3], F32, kind="ExternalInput")
    out_d = nc.dram_tensor("out", [ROWS, N], F32, kind="ExternalOutput")

    AT = mybir.ActivationFunctionType
    OP = mybir.AluOpType

    with tile.TileContext(nc) as tc:
        with tc.tile_pool(name="singles", bufs=1) as singles:
            xt = singles.tile([D, N], BF16)
            xti = singles.tile([D, ROWS], BF16)
            wpack = singles.tile([D, 3 * D + ZW], BF16)
            bpack = singles.tile([D, 3], F32)
            hjsb = singles.tile([D, N], BF16)
            bias = singles.tile([D, ROWS], F32)

            # spread prep loads over several issue queues. sync + vector come
            # up fastest after the NEFF preamble; gpsimd's queue starts ~0.7us
            # later (big post-barrier DRAIN). w1b|w1a and xT gate the prep
            # matmuls -> fastest queues; the rest can land late.
            nc.sync.dma_start(wpack[:, 0:2 * D], wpack_d[:, 0:2 * D])
            nc.scalar.dma_start(xt[:], xT_d[:])
            nc.sync.dma_start(xti[:], xTi_d[:])
            nc.scalar.dma_start(bpack[:], bpack_d[:])
            nc.gpsimd.dma_start(wpack[:, 2 * D:], wpack_d[:, 2 * D:])
            w1b = wpack[:, 0:D]
            w1a = wpack[:, D:2 * D]
            w2d = wpack[:, 2 * D:3 * D]
            zwide = wpack[:, 3 * D:3 * D + ZW]
            b1 = bpack[:, 0:1]
            b2 = bpack[:, 1:2]
            b3 = bpack[:, 2:3]

            # warm ONLY the sigmoid table set under the DMA shadow: relu is a
            # filler function present in every set (incl. sigmoid's), so one
            # load (~2.7us) serves both and no reload happens mid-kernel.
            warm = singles.tile([D, 1], F32)
            nc.vector.memset(warm[:], 0.0)
            nc.scalar.activation(warm[:], warm[:], AT.Sigmoid)

            # ---- prep: hj (bf16) and per-row bias = hi + b1 (fp32) ----
            with tc.tile_pool(name="prep_ps", bufs=1, space="PSUM") as pps:
                hj_ps = pps.tile([D, N], F32)
                nc.tensor.matmul(hj_ps[:], lhsT=w1b[:], rhs=xt[:])
                # cast on DVE: ACT is busy with ~5.3us of table loads (relu +
                # sigmoid warm) at start; keeping the whole stage-1 dependency
                # chain off ACT lets DVE start streaming rows immediately
                nc.vector.tensor_copy(hjsb[:], hj_ps[:])

                # (cast is emitted before the bias op: its dep (hj matmul)
                # lands first, so DVE starts sooner)
                hi_ps = pps.tile([D, ROWS], F32)
                nc.tensor.matmul(hi_ps[:], lhsT=w1a[:], rhs=xti[:])
                nc.vector.tensor_scalar(
                    bias[:], hi_ps[:], b1[:, 0:1], None, OP.add
                )

            # ---- main loop: 3 pairs (6 rows) per iteration. The relu is one
            # [128, 3*512] ACT op spanning 3 psum banks, amortizing ACT's
            # ~290ns fixed per-op cost. Stage-3 matmuls trail by LAG_PAIRS
            # pairs and are emitted in quads (4 disjoint col strips). ----
            PPI = 3
            LAG_PAIRS = 4
            n_it = (PAIRS + PPI - 1) // PPI
            with (
                # h2pool bufs=43 = one tile per iteration, ZERO reuse: the
                # buffer-recycle WAR/WAW edges vanish, so each ACT relu's only
                # dep (ps2 full) rides the inline wait slot -> ~43 fewer
                # EVENT_SEMAPHORE instructions on the critical ACT queue.
                # SBUF: 43*3KB(h2) + 10*6KB(s) + ~9KB rest = ~198KB < 208KB.
                tc.tile_pool(name="spool", bufs=10) as spool,
                tc.tile_pool(name="h2pool", bufs=43) as h2pool,
                tc.tile_pool(name="opool", bufs=2) as opool,
                tc.tile_pool(name="ps2pool", bufs=2, space="PSUM") as ps2pool,
                tc.tile_pool(name="ps3pool", bufs=2, space="PSUM") as ps3pool,
            ):
                state = {"ps3": None, "k_out": 0}
                pending = []  # (h2r tile, pair index within tile)

                def emit_quad():
                    # 4 consecutive processing steps = 4 disjoint 32-col
                    # strips; the 4 matmuls co-start on the PE (4x col
                    # tiling). Strip a accumulates its 16 slots into psum
                    # partitions 32a..32a+32 of the shared ps3 bank.
                    k0 = state["k_out"]
                    assert k0 % 4 == 0
                    if k0 % 64 == 0:
                        state["ps3"] = ps3pool.tile(
                            [D, N], F32, name="ps3", tag="ps3"
                        )
                    ps3 = state["ps3"]
                    slot = (k0 % 64) // 4
                    for a in range(4):
                        h2r_l, kk = pending.pop(0)
                        nc.tensor.matmul(
                            ps3[32 * a:32 * a + 32, :],
                            lhsT=zwide[:, 30 - 2 * slot:ZW - 2 * slot],
                            rhs=h2r_l[:, N * kk:N * kk + N],
                            start=(slot == 0),
                            stop=(slot == 15),
                            skip_group_check=True,
                            # base_partition() caps at 64; strip 3 (96) must
                            # be passed explicitly
                            tile_position=(0, 32 * a),
                        )
                    state["k_out"] = k0 + 4
                    if slot == 15:
                        g = k0 // 64  # 64 pairs = 128 contiguous out rows
                        sig = opool.tile([D, N], F32, tag="sig")
                        if g == PAIRS // 64 - 1:
                            # final group: split sigmoid+DMA into j-halves so
                            # the first half's DMA overlaps the second half's
                            # sigmoid (shorter tail chain)
                            H = N // 2
                            nc.scalar.activation(
                                sig[:, 0:H], ps3[:, 0:H], AT.Sigmoid,
                                bias=b3[:, 0:1], scale=1.0,
                            )
                            nc.sync.dma_start(
                                out_d[D * g:D * g + D, 0:H], sig[:, 0:H])
                            nc.scalar.activation(
                                sig[:, H:N], ps3[:, H:N], AT.Sigmoid,
                                bias=b3[:, 0:1], scale=1.0,
                            )
                            nc.scalar.dma_start(
                                out_d[D * g:D * g + D, H:N], sig[:, H:N])
                        else:
                            nc.scalar.activation(
                                sig[:], ps3[:], AT.Sigmoid,
                                bias=b3[:, 0:1], scale=1.0,
                            )
                            nc.sync.dma_start(out_d[D * g:D * g + D, :], sig[:])

                k_in = 0
                for it in range(n_it):
                    npair = min(PPI, PAIRS - k_in)
                    stile = spool.tile([D, 2 * PPI * N], BF16, tag="s")
                    ss = []
                    for r in range(2 * npair):
                        k = k_in + r // 2
                        i = 2 * _proc_pair(k) + (r % 2)
                        ridx = 2 * k + (r % 2)
                        s = stile[:, r * N:(r + 1) * N]
                        if GP_MOD and ridx % GP_MOD == GP_MOD // 2:
                            # GPSIMD absorbs a slice of stage-1 (runs from
                            # its own queue, overlapping DVE)
                            nc.gpsimd.tensor_scalar(
                                s, hjsb[:], bias[:, i:i + 1], 0.0, OP.add, OP.max
                            )
                        elif ACT_S1_MOD and ridx % ACT_S1_MOD == ACT_S1_MOD // 2:
                            nc.scalar.activation(
                                s, hjsb[:], AT.Relu,
                                bias=bias[:, i:i + 1], scale=1.0,
                            )
                        else:
                            nc.vector.tensor_scalar(
                                s, hjsb[:], bias[:, i:i + 1], 0.0, OP.add, OP.max
                            )
                        ss.append(s)

                    ps2 = ps2pool.tile([D, PPI * N], F32)
                    for k in range(npair):
                        nc.tensor.matmul(
                            ps2[0:64, k * N:(k + 1) * N],
                            lhsT=w2d[:, 0:64], rhs=ss[2 * k])
                        nc.tensor.matmul(
                            ps2[64:128, k * N:(k + 1) * N],
                            lhsT=w2d[:, 64:128], rhs=ss[2 * k + 1])

                    h2r = h2pool.tile([D, PPI * N], BF16, tag="h2r")
                    if it >= DVE_RELU_FROM:
                        # tail relus on DVE (PSUM src fp32 -> 1x) ride its
                        # post-stage-1 idle window
                        nc.vector.tensor_scalar(
                            h2r[:, 0:npair * N], ps2[:, 0:npair * N],
                            b2[:, 0:1], 0.0, OP.add, OP.max,
                        )
                    else:
                        nc.scalar.activation(
                            h2r[:, 0:npair * N], ps2[:, 0:npair * N], AT.Relu,
                            bias=b2[:, 0:1], scale=1.0,
                        )
                    for k in range(npair):
                        pending.append((h2r, k))
                    k_in += npair

                    while len(pending) >= 4 + LAG_PAIRS:
                        emit_quad()
                while pending:
                    emit_quad()
    nc.finalize()
    return nc


_CACHED_NC = None


def _get_nc():
    global _CACHED_NC
    if _CACHED_NC is None:
        _CACHED_NC = build_nc()
    return _CACHED_NC


def _host_prep(node_features, W1, b1, W2, b2, W3, b3):
    bf = ml_dtypes.bfloat16
    w1a = W1[:D]
    w1b = W1[D:]
    w2d = np.concatenate([W2, W2], axis=1)
    zwide = np.zeros((D, ZW), np.float32)
    zwide[0:64, 30] = W3[:, 0]
    zwide[64:128, 31] = W3[:, 0]
    wpack = np.ascontiguousarray(
        np.concatenate([w1b, w1a, w2d, zwide], axis=1).astype(bf))
    bpack = np.ascontiguousarray(np.stack([
        b1, np.concatenate([b2, b2]), np.full(D, b3[0])
    ], axis=1).astype(np.float32))

    in_maps = []
    for c in range(NCORES):
        b, half = c // 2, c % 2
        xT = np.ascontiguousarray(node_features[b].T.astype(bf))
        xTi = np.ascontiguousarray(xT[:, half * ROWS:(half + 1) * ROWS])
        in_maps.append({
            "xT": xT, "xTi": xTi, "wpack": wpack, "bpack": bpack,
        })
    return in_maps


def run(node_features, W1, b1, W2, b2, W3, b3, **spmd_kwargs):
    """Run the bass kernel; returns (full_output, BassKernelResults)."""
    nc = _get_nc()
    in_maps = _host_prep(node_features, W1, b1, W2, b2, W3, b3)
    res = run_bass_kernel_spmd(nc, in_maps, core_ids=list(range(NCORES)), **spmd_kwargs)
    out = np.empty((B, N, N), np.float32)
    for c in range(NCORES):
        b, half = c // 2, c % 2
        out[b, half * ROWS:(half + 1) * ROWS, :] = res.results[c]["out"]
    return out, res


def kernel(node_features, W1, b1, W2, b2, W3, b3):
    out, _ = run(node_features, W1, b1, W2, b2, W3, b3)
    return out


# revision 24
# speedup vs baseline: 1.0047x; 1.0047x over previous
"""Trainium2 Bass kernel for the CoOccurrenceEncoder pairwise-MLP problem.

Reference computation (per batch b of 4, N=512 nodes, d=128):
    hi = x @ W1[:d]          # [N, d]
    hj = x @ W1[d:]          # [N, d]
    h  = relu(hi[:,None,:] + hj[None,:,:] + b1)       # [N, N, d]
    h2 = relu(h @ W2 + b2)                            # [N, N, 64]
    out = sigmoid((h2 @ W3 + b3)[..., 0])             # [N, N]

Sharding: 8 cores; core c handles batch c//2, i-rows [256*(c%2), 256*(c%2)+256).
Each core holds its batch's full hj [d=128 partitions, N=512] in SBUF (bf16) and
streams 256 i-rows; weights are tiny and replicated.

Per-core dataflow (d=128 on partitions everywhere):
  stage1 (DVE, bf16 2x_1P): S_i = relu(hj + (hi_i + b1)) via one dual-op
          tensor_scalar (add per-partition vector, max 0) per row -> SBUF bf16.
          Every GP_MOD-th row runs on GPSIMD instead to offload DVE.
  stage2 (PE): stationary [W2 | W2] (128x128 bf16); a row PAIR runs as two
          column-tiled matmuls that co-start -> h2 fp32 [128, 2*512] PSUM
  stage2b (ACT): relu(h2 + b2) PSUM -> SBUF bf16 (1x, PSUM-source bound);
          every DVE_RELU_PERIOD-th iteration's relu runs on DVE instead.
  stage3 (PE, 4x column-tiled): pair processed at step k (within a 64-pair
          group) targets col strip k%4 (psum partitions 32*(k%4)..+32), slot
          t=k//4 via the 32-wide stationary window zwide[:, 30-2t:62-2t]
          whose W3 pair sits at strip-relative cols 2t,2t+1. Rows are
          PERMUTED so that the pair processed at step k is pi(k) =
          16*(k%4) + k//4 -- its logits land exactly at psum partitions
          2*pi(k), 2*pi(k)+1, i.e. the identity layout. Four consecutive
          stage-3 matmuls hit 4 disjoint strips and co-start on the PE
          (4x col tiling), quartering stage-3 PE time vs the 128-wide
          zero-padded scheme.
  stage4 (ACT): one sigmoid(logits + b3) [128,512] per 64 pairs -> one
          contiguous [128,512] HWDGE DMA to HBM.
"""

import numpy as np
import ml_dtypes

import concourse.bass as bass
import concourse.mybir as mybir
import concourse.tile as tile
from concourse import bacc
from concourse.bass_utils import run_bass_kernel_spmd

F32 = mybir.dt.float32
BF16 = mybir.dt.bfloat16

D = 128          # feature dim (= partitions)
N = 512          # nodes per batch
B = 4            # batches
NCORES = 8
ROWS = 256       # i-rows per core
PAIRS = ROWS // 2
ZW = 62          # zwide stationary width

# iterations >= DVE_RELU_FROM run their stage2-relu on DVE instead of ACT.
# Mid-stream DVE relus head-of-line-block stage-1 rows (measured 10us PE
# stalls), but TAIL iterations are safe: DVE finishes its 256 stage-1 rows
# ~13us before ACT drains, so the last few relus ride DVE's idle tail and
# directly shorten the ACT critical path (~1.54us each).
# MEASURED: each non-final DVE relu precedes later iterations' stage-1 rows
# in the DVE FIFO and waits on PE mid-stream -> pushes DVE's stream end out
# ~2-3us apiece (FROM=38 regressed to 93.8us). Only the last iteration's
# relu (after ALL rows) rides the idle tail cleanly.
DVE_RELU_FROM = 42
# stage-1 rows with ridx % GP_MOD == GP_MOD//2 run on GPSIMD (0 = none).
# MEASURED: gpsimd tensor_scalar [128,512] = 7530ns steady-state AND its SBUF
# port contention doubles concurrent DVE op durations. Unusable.
GP_MOD = 0
# stage-1 rows with ridx % ACT_S1_MOD == ACT_S1_MOD//2 run on ACT (0 = none).
ACT_S1_MOD = 0


def _proc_pair(k):
    """Pair processed at step k: 64-pair groups, strip-interleaved so 4
    consecutive stage-3 matmuls target 4 disjoint 32-col PE strips."""
    g, k2 = divmod(k, 64)
    return 64 * g + 16 * (k2 % 4) + k2 // 4


def build_nc():
    # Bacc (not plain Bass): its compile() runs move_matmul_waits_to_ldweights
    # + generate_event_semaphores, needed to satisfy TRN2's 1-wait-per-matmul
    # hardware constraint.
    nc = bacc.Bacc("TRN2")

    # wpack = [w1b | w1a | w2dup | zwide] along free; bpack = [b1 | b2dup | b3]
    # zwide [128, 62]: col 30 = [W3;0], col 31 = [0;W3], zeros elsewhere. The
    # stage-3 stationary for slot t is the 32-col window zwide[:, 30-2t:62-2t]:
    # its W3 pair lands at strip-relative cols 2t,2t+1 and every other column
    # is zero, so each matmul writes its whole 32-partition strip (slot 0 with
    # start=True clears that strip's has_written bits; later slots accumulate
    # +0 everywhere except their own 2 partitions).
    xT_d = nc.dram_tensor("xT", [D, N], BF16, kind="ExternalInput")
    xTi_d = nc.dram_tensor("xTi", [D, ROWS], BF16, kind="ExternalInput")
    wpack_d = nc.dram_tensor("wpack", [D, 3 * D + ZW], BF16, kind="ExternalInput")
    bpack_d = nc.dram_tensor("bpack", [D, 3], F32, kind="ExternalInput")
    out_d = nc.dram_tensor("out", [ROWS, N], F32, kind="ExternalOutput")

    AT = mybir.ActivationFunctionType
    OP = mybir.AluOpType

    with tile.TileContext(nc) as tc:
        with tc.tile_pool(name="singles", bufs=1) as singles:
            xt = singles.tile([D, N], BF16)
            xti = singles.tile([D, ROWS], BF16)
            wpack = singles.tile([D, 3 * D + ZW], BF16)
            bpack = singles.tile([D, 3], F32)
            hjsb = singles.tile([D, N], BF16)
            bias = singles.tile([D, ROWS], F32)

            # spread prep loads over several issue queues. sync + vector come
            # up fastest after the NEFF preamble; gpsimd's queue starts ~0.7us
            # later (big post-barrier DRAIN). w1b|w1a and xT gate the prep
            # matmuls -> fastest queues; the rest can land late.
            nc.sync.dma_start(wpack[:, 0:2 * D], wpack_d[:, 0:2 * D])
            nc.scalar.dma_start(xt[:], xT_d[:])
            nc.sync.dma_start(xti[:], xTi_d[:])
            nc.scalar.dma_start(bpack[:], bpack_d[:])
            nc.gpsimd.dma_start(wpack[:, 2 * D:], wpack_d[:, 2 * D:])
            w1b = wpack[:, 0:D]
            w1a = wpack[:, D:2 * D]
            w2d = wpack[:, 2 * D:3 * D]
            zwide = wpack[:, 3 * D:3 * D + ZW]
            b1 = bpack[:, 0:1]
            b2 = bpack[:, 1:2]
            b3 = bpack[:, 2:3]

            # warm ONLY the sigmoid table set under the DMA shadow: relu is a
            # filler function present in every set (incl. sigmoid's), so one
            # load (~2.7us) serves both and no reload happens mid-kernel.
            warm = singles.tile([D, 1], F32)
            nc.vector.memset(warm[:], 0.0)
            nc.scalar.activation(warm[:], warm[:], AT.Sigmoid)

            # ---- prep: hj (bf16) and per-row bias = hi + b1 (fp32) ----
            with tc.tile_pool(name="prep_ps", bufs=1, space="PSUM") as pps:
                hj_ps = pps.tile([D, N], F32)
                nc.tensor.matmul(hj_ps[:], lhsT=w1b[:], rhs=xt[:])
                # cast on DVE: ACT is busy with ~5.3us of table loads (relu +
                # sigmoid warm) at start; keeping the whole stage-1 dependency
                # chain off ACT lets DVE start streaming rows immediately
                nc.vector.tensor_copy(hjsb[:], hj_ps[:])

                # (cast is emitted before the bias op: its dep (hj matmul)
                # lands first, so DVE starts sooner)
                hi_ps = pps.tile([D, ROWS], F32)
                nc.tensor.matmul(hi_ps[:], lhsT=w1a[:], rhs=xti[:])
                nc.vector.tensor_scalar(
                    bias[:], hi_ps[:], b1[:, 0:1], None, OP.add
                )

            # ---- main loop: 3 pairs (6 rows) per iteration. The relu is one
            # [128, 3*512] ACT op spanning 3 psum banks, amortizing ACT's
            # ~290ns fixed per-op cost. Stage-3 matmuls trail by LAG_PAIRS
            # pairs and are emitted in quads (4 disjoint col strips). ----
            PPI = 3
            LAG_PAIRS = 4
            n_it = (PAIRS + PPI - 1) // PPI
            with (
                # h2pool bufs=43 = one tile per iteration, ZERO reuse: the
                # buffer-recycle WAR/WAW edges vanish, so each ACT relu's only
                # dep (ps2 full) rides the inline wait slot -> ~43 fewer
                # EVENT_SEMAPHORE instructions on the critical ACT queue.
                # SBUF: 43*3KB(h2) + 10*6KB(s) + ~9KB rest = ~198KB < 208KB.
                tc.tile_pool(name="spool", bufs=10) as spool,
                tc.tile_pool(name="h2pool", bufs=43) as h2pool,
                tc.tile_pool(name="opool", bufs=2) as opool,
                tc.tile_pool(name="ps2pool", bufs=2, space="PSUM") as ps2pool,
                tc.tile_pool(name="ps3pool", bufs=2, space="PSUM") as ps3pool,
            ):
                state = {"ps3": None, "k_out": 0}
                pending = []  # (h2r tile, pair index within tile)

                def emit_quad():
                    # 4 consecutive processing steps = 4 disjoint 32-col
                    # strips; the 4 matmuls co-start on the PE (4x col
                    # tiling). Strip a accumulates its 16 slots into psum
                    # partitions 32a..32a+32 of the shared ps3 bank.
                    k0 = state["k_out"]
                    assert k0 % 4 == 0
                    if k0 % 64 == 0:
                        state["ps3"] = ps3pool.tile(
                            [D, N], F32, name="ps3", tag="ps3"
                        )
                    ps3 = state["ps3"]
                    slot = (k0 % 64) // 4
                    for a in range(4):
                        h2r_l, kk = pending.pop(0)
                        nc.tensor.matmul(
                            ps3[32 * a:32 * a + 32, :],
                            lhsT=zwide[:, 30 - 2 * slot:ZW - 2 * slot],
                            rhs=h2r_l[:, N * kk:N * kk + N],
                            start=(slot == 0),
                            stop=(slot == 15),
                            skip_group_check=True,
                            # base_partition() caps at 64; strip 3 (96) must
                            # be passed explicitly
                            tile_position=(0, 32 * a),
                        )
                    state["k_out"] = k0 + 4
                    if slot == 15:
                        g = k0 // 64  # 64 pairs = 128 contiguous out rows
                        sig = opool.tile([D, N], F32, tag="sig")
                        nc.scalar.activation(
                            sig[:], ps3[:], AT.Sigmoid, bias=b3[:, 0:1], scale=1.0
                        )
                        nc.sync.dma_start(out_d[D * g:D * g + D, :], sig[:])

                k_in = 0
                for it in range(n_it):
                    npair = min(PPI, PAIRS - k_in)
                    stile = spool.tile([D, 2 * PPI * N], BF16, tag="s")
                    ss = []
                    for r in range(2 * npair):
                        k = k_in + r // 2
                        i = 2 * _proc_pair(k) + (r % 2)
                        ridx = 2 * k + (r % 2)
                        s = stile[:, r * N:(r + 1) * N]
                        if GP_MOD and ridx % GP_MOD == GP_MOD // 2:
                            # GPSIMD absorbs a slice of stage-1 (runs from
                            # its own queue, overlapping DVE)
                            nc.gpsimd.tensor_scalar(
                                s, hjsb[:], bias[:, i:i + 1], 0.0, OP.add, OP.max
                            )
                        elif ACT_S1_MOD and ridx % ACT_S1_MOD == ACT_S1_MOD // 2:
                            nc.scalar.activation(
                                s, hjsb[:], AT.Relu,
                                bias=bias[:, i:i + 1], scale=1.0,
                            )
                        else:
                            nc.vector.tensor_scalar(
                                s, hjsb[:], bias[:, i:i + 1], 0.0, OP.add, OP.max
                            )
                        ss.append(s)

                    ps2 = ps2pool.tile([D, PPI * N], F32)
                    for k in range(npair):
                        nc.tensor.matmul(
                            ps2[0:64, k * N:(k + 1) * N],
                            lhsT=w2d[:, 0:64], rhs=ss[2 * k])
                        nc.tensor.matmul(
                            ps2[64:128, k * N:(k + 1) * N],
                            lhsT=w2d[:, 64:128], rhs=ss[2 * k + 1])

                    h2r = h2pool.tile([D, PPI * N], BF16, tag="h2r")
                    if it >= DVE_RELU_FROM:
                        # tail relus on DVE (PSUM src fp32 -> 1x) ride its
                        # post-stage-1 idle window
                        nc.vector.tensor_scalar(
                            h2r[:, 0:npair * N], ps2[:, 0:npair * N],
                            b2[:, 0:1], 0.0, OP.add, OP.max,
                        )
                    else:
                        nc.scalar.activation(
                            h2r[:, 0:npair * N], ps2[:, 0:npair * N], AT.Relu,
                            bias=b2[:, 0:1], scale=1.0,
                        )
                    for k in range(npair):
                        pending.append((h2r, k))
                    k_in += npair

                    while len(pending) >= 4 + LAG_PAIRS:
                        emit_quad()
                while pending:
                    emit_quad()
    nc.finalize()
    return nc


_CACHED_NC = None


def _get_nc():
    global _CACHED_NC
    if _CACHED_NC is None:
        _CACHED_NC = build_nc()
    return _CACHED_NC


def _host_prep(node_features, W1, b1, W2, b2, W3, b3):
    bf = ml_dtypes.bfloat16
    w1a = W1[:D]
    w1b = W1[D:]
    w2d = np.concatenate([W2, W2], axis=1)
    zwide = np.zeros((D, ZW), np.float32)
    zwide[0:64, 30] = W3[:, 0]
    zwide[64:128, 31] = W3[:, 0]
    wpack = np.ascontiguousarray(
        np.concatenate([w1b, w1a, w2d, zwide], axis=1).astype(bf))
    bpack = np.ascontiguousarray(np.stack([
        b1, np.concatenate([b2, b2]), np.full(D, b3[0])
    ], axis=1).astype(np.float32))

    in_maps = []
    for c in range(NCORES):
        b, half = c // 2, c % 2
        xT = np.ascontiguousarray(node_features[b].T.astype(bf))
        xTi = np.ascontiguousarray(xT[:, half * ROWS:(half + 1) * ROWS])
        in_maps.append({
            "xT": xT, "xTi": xTi, "wpack": wpack, "bpack": bpack,
        })
    return in_maps


def run(node_features, W1, b1, W2, b2, W3, b3, **spmd_kwargs):
    """Run the bass kernel; returns (full_output, BassKernelResults)."""
    nc = _get_nc()
    in_maps = _host_prep(node_features, W1, b1, W2, b2, W3, b3)
    res = run_bass_kernel_spmd(nc, in_maps, core_ids=list(range(NCORES)), **spmd_kwargs)
    out = np.empty((B, N, N), np.float32)
    for c in range(NCORES):
        b, half = c // 2, c % 2
        out[b, half * ROWS:(half + 1) * ROWS, :] = res.results[c]["out"]
    return out, res


def kernel(node_features, W1, b1, W2, b2, W3, b3):
    out, _ = run(node_features, W1, b1, W2, b2, W3, b3)
    return out


# revision 27
# speedup vs baseline: 1.0076x; 1.0029x over previous
"""Trainium2 Bass kernel for the CoOccurrenceEncoder pairwise-MLP problem.

Reference computation (per batch b of 4, N=512 nodes, d=128):
    hi = x @ W1[:d]          # [N, d]
    hj = x @ W1[d:]          # [N, d]
    h  = relu(hi[:,None,:] + hj[None,:,:] + b1)       # [N, N, d]
    h2 = relu(h @ W2 + b2)                            # [N, N, 64]
    out = sigmoid((h2 @ W3 + b3)[..., 0])             # [N, N]

Sharding: 8 cores; core c handles batch c//2, i-rows [256*(c%2), 256*(c%2)+256).
Each core holds its batch's full hj [d=128 partitions, N=512] in SBUF (bf16) and
streams 256 i-rows; weights are tiny and replicated.

Per-core dataflow (d=128 on partitions everywhere):
  stage1 (DVE, bf16 2x_1P): S_i = relu(hj + (hi_i + b1)) via one dual-op
          tensor_scalar (add per-partition vector, max 0) per row -> SBUF bf16.
          Every GP_MOD-th row runs on GPSIMD instead to offload DVE.
  stage2 (PE): stationary [W2 | W2] (128x128 bf16); a row PAIR runs as two
          column-tiled matmuls that co-start -> h2 fp32 [128, 2*512] PSUM
  stage2b (ACT): relu(h2 + b2) PSUM -> SBUF bf16 (1x, PSUM-source bound);
          every DVE_RELU_PERIOD-th iteration's relu runs on DVE instead.
  stage3 (PE, 4x column-tiled): pair processed at step k (within a 64-pair
          group) targets col strip k%4 (psum partitions 32*(k%4)..+32), slot
          t=k//4 via the 32-wide stationary window zwide[:, 30-2t:62-2t]
          whose W3 pair sits at strip-relative cols 2t,2t+1. Rows are
          PERMUTED so that the pair processed at step k is pi(k) =
          16*(k%4) + k//4 -- its logits land exactly at psum partitions
          2*pi(k), 2*pi(k)+1, i.e. the identity layout. Four consecutive
          stage-3 matmuls hit 4 disjoint strips and co-start on the PE
          (4x col tiling), quartering stage-3 PE time vs the 128-wide
          zero-padded scheme.
  stage4 (ACT): one sigmoid(logits + b3) [128,512] per 64 pairs -> one
          contiguous [128,512] HWDGE DMA to HBM.
"""

import numpy as np
import ml_dtypes

import concourse.bass as bass
import concourse.mybir as mybir
import concourse.tile as tile
from concourse import bacc
from concourse.bass_utils import run_bass_kernel_spmd

F32 = mybir.dt.float32
BF16 = mybir.dt.bfloat16

D = 128          # feature dim (= partitions)
N = 512          # nodes per batch
B = 4            # batches
NCORES = 8
ROWS = 256       # i-rows per core
PAIRS = ROWS // 2
ZW = 62          # zwide stationary width

# iterations >= DVE_RELU_FROM run their stage2-relu on DVE instead of ACT.
# Mid-stream DVE relus head-of-line-block stage-1 rows (measured 10us PE
# stalls), but TAIL iterations are safe: DVE finishes its 256 stage-1 rows
# ~13us before ACT drains, so the last few relus ride DVE's idle tail and
# directly shorten the ACT critical path (~1.54us each).
# MEASURED: each non-final DVE relu precedes later iterations' stage-1 rows
# in the DVE FIFO and waits on PE mid-stream -> pushes DVE's stream end out
# ~2-3us apiece (FROM=38 regressed to 93.8us). Only the last iteration's
# relu (after ALL rows) rides the idle tail cleanly.
DVE_RELU_FROM = 42
# stage-1 rows with ridx % GP_MOD == GP_MOD//2 run on GPSIMD (0 = none).
# MEASURED: gpsimd tensor_scalar [128,512] = 7530ns steady-state AND its SBUF
# port contention doubles concurrent DVE op durations. Unusable.
GP_MOD = 0
# stage-1 rows with ridx % ACT_S1_MOD == ACT_S1_MOD//2 run on ACT (0 = none).
ACT_S1_MOD = 0
# from this iteration on, quad emission is deferred to the post-loop drain:
# the tail quads wait on late ACT relus, and emitting them BEFORE the last
# stage2 matmuls head-of-line-blocks the PE FIFO -> the final DVE relu
# (measured) starts ~3.9us late. Deferring lets all remaining stage2 issue
# first, so the relu->final-quad->sigmoid->DMA chain starts immediately.
DEFER_QUADS_FROM = 40


def _proc_pair(k):
    """Pair processed at step k: 64-pair groups, strip-interleaved so 4
    consecutive stage-3 matmuls target 4 disjoint 32-col PE strips."""
    g, k2 = divmod(k, 64)
    return 64 * g + 16 * (k2 % 4) + k2 // 4


def build_nc():
    # Bacc (not plain Bass): its compile() runs move_matmul_waits_to_ldweights
    # + generate_event_semaphores, needed to satisfy TRN2's 1-wait-per-matmul
    # hardware constraint.
    nc = bacc.Bacc("TRN2")

    # wpack = [w1b | w1a | w2dup | zwide] along free; bpack = [b1 | b2dup | b3]
    # zwide [128, 62]: col 30 = [W3;0], col 31 = [0;W3], zeros elsewhere. The
    # stage-3 stationary for slot t is the 32-col window zwide[:, 30-2t:62-2t]:
    # its W3 pair lands at strip-relative cols 2t,2t+1 and every other column
    # is zero, so each matmul writes its whole 32-partition strip (slot 0 with
    # start=True clears that strip's has_written bits; later slots accumulate
    # +0 everywhere except their own 2 partitions).
    xT_d = nc.dram_tensor("xT", [D, N], BF16, kind="ExternalInput")
    xTi_d = nc.dram_tensor("xTi", [D, ROWS], BF16, kind="ExternalInput")
    wpack_d = nc.dram_tensor("wpack", [D, 3 * D + ZW], BF16, kind="ExternalInput")
    bpack_d = nc.dram_tensor("bpack", [D, 3], F32, kind="ExternalInput")
    out_d = nc.dram_tensor("out", [ROWS, N], F32, kind="ExternalOutput")

    AT = mybir.ActivationFunctionType
    OP = mybir.AluOpType

    with tile.TileContext(nc) as tc:
        with tc.tile_pool(name="singles", bufs=1) as singles:
            xt = singles.tile([D, N], BF16)
            xti = singles.tile([D, ROWS], BF16)
            wpack = singles.tile([D, 3 * D + ZW], BF16)
            bpack = singles.tile([D, 3], F32)
            hjsb = singles.tile([D, N], BF16)
            bias = singles.tile([D, ROWS], F32)

            # spread prep loads over several issue queues. sync + vector come
            # up fastest after the NEFF preamble; gpsimd's queue starts ~0.7us
            # later (big post-barrier DRAIN). w1b|w1a and xT gate the prep
            # matmuls -> fastest queues; the rest can land late.
            nc.sync.dma_start(wpack[:, 0:2 * D], wpack_d[:, 0:2 * D])
            nc.scalar.dma_start(xt[:], xT_d[:])
            nc.sync.dma_start(xti[:], xTi_d[:])
            nc.scalar.dma_start(bpack[:], bpack_d[:])
            nc.gpsimd.dma_start(wpack[:, 2 * D:], wpack_d[:, 2 * D:])
            w1b = wpack[:, 0:D]
            w1a = wpack[:, D:2 * D]
            w2d = wpack[:, 2 * D:3 * D]
            zwide = wpack[:, 3 * D:3 * D + ZW]
            b1 = bpack[:, 0:1]
            b2 = bpack[:, 1:2]
            b3 = bpack[:, 2:3]

            # warm ONLY the sigmoid table set under the DMA shadow: relu is a
            # filler function present in every set (incl. sigmoid's), so one
            # load (~2.7us) serves both and no reload happens mid-kernel.
            warm = singles.tile([D, 1], F32)
            nc.vector.memset(warm[:], 0.0)
            nc.scalar.activation(warm[:], warm[:], AT.Sigmoid)

            # ---- prep: hj (bf16) and per-row bias = hi + b1 (fp32) ----
            with tc.tile_pool(name="prep_ps", bufs=1, space="PSUM") as pps:
                hj_ps = pps.tile([D, N], F32)
                nc.tensor.matmul(hj_ps[:], lhsT=w1b[:], rhs=xt[:])
                # cast on ACT: its table loads finish (~+10.1us) just before
                # hj_ps is ready (~+10.2us), so the cast is free there and DVE
                # only has the bias op before its first stage-1 row
                nc.scalar.activation(hjsb[:], hj_ps[:], AT.Copy)

                # (cast is emitted before the bias op: its dep (hj matmul)
                # lands first, so DVE starts sooner)
                hi_ps = pps.tile([D, ROWS], F32)
                nc.tensor.matmul(hi_ps[:], lhsT=w1a[:], rhs=xti[:])
                nc.vector.tensor_scalar(
                    bias[:], hi_ps[:], b1[:, 0:1], None, OP.add
                )

            # ---- main loop: 3 pairs (6 rows) per iteration. The relu is one
            # [128, 3*512] ACT op spanning 3 psum banks, amortizing ACT's
            # ~290ns fixed per-op cost. Stage-3 matmuls trail by LAG_PAIRS
            # pairs and are emitted in quads (4 disjoint col strips). ----
            PPI = 3
            LAG_PAIRS = 4
            n_it = (PAIRS + PPI - 1) // PPI
            with (
                # h2pool bufs=43 = one tile per iteration, ZERO reuse: the
                # buffer-recycle WAR/WAW edges vanish, so each ACT relu's only
                # dep (ps2 full) rides the inline wait slot -> ~43 fewer
                # EVENT_SEMAPHORE instructions on the critical ACT queue.
                # SBUF: 43*3KB(h2) + 10*6KB(s) + ~9KB rest = ~198KB < 208KB.
                tc.tile_pool(name="spool", bufs=10) as spool,
                tc.tile_pool(name="h2pool", bufs=43) as h2pool,
                tc.tile_pool(name="opool", bufs=2) as opool,
                tc.tile_pool(name="ps2pool", bufs=2, space="PSUM") as ps2pool,
                tc.tile_pool(name="ps3pool", bufs=2, space="PSUM") as ps3pool,
            ):
                state = {"ps3": None, "k_out": 0}
                pending = []  # (h2r tile, pair index within tile)

                def emit_quad():
                    # 4 consecutive processing steps = 4 disjoint 32-col
                    # strips; the 4 matmuls co-start on the PE (4x col
                    # tiling). Strip a accumulates its 16 slots into psum
                    # partitions 32a..32a+32 of the shared ps3 bank.
                    k0 = state["k_out"]
                    assert k0 % 4 == 0
                    if k0 % 64 == 0:
                        state["ps3"] = ps3pool.tile(
                            [D, N], F32, name="ps3", tag="ps3"
                        )
                    ps3 = state["ps3"]
                    slot = (k0 % 64) // 4
                    for a in range(4):
                        h2r_l, kk = pending.pop(0)
                        nc.tensor.matmul(
                            ps3[32 * a:32 * a + 32, :],
                            lhsT=zwide[:, 30 - 2 * slot:ZW - 2 * slot],
                            rhs=h2r_l[:, N * kk:N * kk + N],
                            start=(slot == 0),
                            stop=(slot == 15),
                            skip_group_check=True,
                            # base_partition() caps at 64; strip 3 (96) must
                            # be passed explicitly
                            tile_position=(0, 32 * a),
                        )
                    state["k_out"] = k0 + 4
                    if slot == 15:
                        g = k0 // 64  # 64 pairs = 128 contiguous out rows
                        sig = opool.tile([D, N], F32, tag="sig")
                        nc.scalar.activation(
                            sig[:], ps3[:], AT.Sigmoid, bias=b3[:, 0:1], scale=1.0
                        )
                        nc.sync.dma_start(out_d[D * g:D * g + D, :], sig[:])

                k_in = 0
                for it in range(n_it):
                    npair = min(PPI, PAIRS - k_in)
                    stile = spool.tile([D, 2 * PPI * N], BF16, tag="s")
                    ss = []
                    for r in range(2 * npair):
                        k = k_in + r // 2
                        i = 2 * _proc_pair(k) + (r % 2)
                        ridx = 2 * k + (r % 2)
                        s = stile[:, r * N:(r + 1) * N]
                        if GP_MOD and ridx % GP_MOD == GP_MOD // 2:
                            # GPSIMD absorbs a slice of stage-1 (runs from
                            # its own queue, overlapping DVE)
                            nc.gpsimd.tensor_scalar(
                                s, hjsb[:], bias[:, i:i + 1], 0.0, OP.add, OP.max
                            )
                        elif ACT_S1_MOD and ridx % ACT_S1_MOD == ACT_S1_MOD // 2:
                            nc.scalar.activation(
                                s, hjsb[:], AT.Relu,
                                bias=bias[:, i:i + 1], scale=1.0,
                            )
                        else:
                            nc.vector.tensor_scalar(
                                s, hjsb[:], bias[:, i:i + 1], 0.0, OP.add, OP.max
                            )
                        ss.append(s)

                    ps2 = ps2pool.tile([D, PPI * N], F32)
                    for k in range(npair):
                        nc.tensor.matmul(
                            ps2[0:64, k * N:(k + 1) * N],
                            lhsT=w2d[:, 0:64], rhs=ss[2 * k])
                        nc.tensor.matmul(
                            ps2[64:128, k * N:(k + 1) * N],
                            lhsT=w2d[:, 64:128], rhs=ss[2 * k + 1])

                    h2r = h2pool.tile([D, PPI * N], BF16, tag="h2r")
                    if it >= DVE_RELU_FROM:
                        # tail relus on DVE (PSUM src fp32 -> 1x) ride its
                        # post-stage-1 idle window
                        nc.vector.tensor_scalar(
                            h2r[:, 0:npair * N], ps2[:, 0:npair * N],
                            b2[:, 0:1], 0.0, OP.add, OP.max,
                        )
                    else:
                        nc.scalar.activation(
                            h2r[:, 0:npair * N], ps2[:, 0:npair * N], AT.Relu,
                            bias=b2[:, 0:1], scale=1.0,
                        )
                    for k in range(npair):
                        pending.append((h2r, k))
                    k_in += npair

                    if it < DEFER_QUADS_FROM:
                        while len(pending) >= 4 + LAG_PAIRS:
                            emit_quad()
                while pending:
                    emit_quad()
    nc.finalize()
    return nc


_CACHED_NC = None


def _get_nc():
    global _CACHED_NC
    if _CACHED_NC is None:
        _CACHED_NC = build_nc()
    return _CACHED_NC


def _host_prep(node_features, W1, b1, W2, b2, W3, b3):
    bf = ml_dtypes.bfloat16
    w1a = W1[:D]
    w1b = W1[D:]
    w2d = np.concatenate([W2, W2], axis=1)
    zwide = np.zeros((D, ZW), np.float32)
    zwide[0:64, 30] = W3[:, 0]
    zwide[64:128, 31] = W3[:, 0]
    wpack = np.ascontiguousarray(
        np.concatenate([w1b, w1a, w2d, zwide], axis=1).astype(bf))
    bpack = np.ascontiguousarray(np.stack([
        b1, np.concatenate([b2, b2]), np.full(D, b3[0])
    ], axis=1).astype(np.float32))

    in_maps = []
    for c in range(NCORES):
        b, half = c // 2, c % 2
        xT = np.ascontiguousarray(node_features[b].T.astype(bf))
        xTi = np.ascontiguousarray(xT[:, half * ROWS:(half + 1) * ROWS])
        in_maps.append({
            "xT": xT, "xTi": xTi, "wpack": wpack, "bpack": bpack,
        })
    return in_maps


def run(node_features, W1, b1, W2, b2, W3, b3, **spmd_kwargs):
    """Run the bass kernel; returns (full_output, BassKernelResults)."""
    nc = _get_nc()
    in_maps = _host_prep(node_features, W1, b1, W2, b2, W3, b3)
    res = run_bass_kernel_spmd(nc, in_maps, core_ids=list(range(NCORES)), **spmd_kwargs)
    out = np.empty((B, N, N), np.float32)
    for c in range(NCORES):
        b, half = c // 2, c % 2
        out[b, half * ROWS:(half + 1) * ROWS, :] = res.results[c]["out"]
    return out, res


def kernel(node_features, W1, b1, W2, b2, W3, b3):
    out, _ = run(node_features, W1, b1, W2, b2, W3, b3)
    return out
